# revision 1
# baseline (speedup 1.0000x reference)
"""CrossNetMix (DCN-V2 MoE cross-network) Trainium2 Bass kernel.

Math (per layer i, reference):
    v    = tanh(x_l @ V[i])      per expert      (B, E, R)
    c    = tanh(v @ C[i].T)      per expert      (B, E, R)
    u    = c @ U[i].T            per expert      (B, E, D)
    gate = softmax(x_l @ G.T)                    (B, E)
    x_l  = sum_e gate_e * x0 * (u_e + bias[i]) + x_l

Since softmax gates sum to 1 the update is
    x_{l+1} = x0 * (sum_e gate_e u_e + bias[i]) + x_l
and with S_0 = 1, x_l = x0 * S_l where
    S_{l+1} = S_l + umix_l + bias[i],   umix = U_arr^T (gate256 * c)

Device layout: features on partitions, tokens on the free dim.  The host
pre-transposes each core's x slice to (D, Bc) so every DMA is contiguous
and the whole matmul chain (V -> C -> U) stays feature-major with zero
on-device transposes.  Gate softmax over the E=4 partition dim uses a
GPSIMD partition all-reduce (Pool engine, otherwise idle) and a selector
matmul for the 4->256 broadcast.

The S recurrence runs through PSUM *prewrites*: define P_l = S_l + b_l.
The (otherwise idle) Pool engine copies P_l into the U-matmul psum bank
before the U matmuls accumulate onto it (start=False), so the psum ends
as P_l + umix_l = S_{l+1}, and
  - x_{l+1} = psum * x0 is one DVE tensor_tensor,
  - P_{l+1} = psum + b_{l+1} is one ACT copy with a fused per-partition
    bias column.
Layer 0 needs no prewrite: P_0 = 1 + b_0 is a per-partition constant that
rides the DVE scalar_tensor_tensor / ACT bias operands instead.  This
removes all identity-matmul accumulation passes from the PE.

DMAs are batched (one per weight-layer, one per token-block for x/out) to
keep HWDGE descriptor-generation off the critical path.
"""

import numpy as np

import concourse.bacc as bacc
import concourse.bass as bass
import concourse.bass_isa as bass_isa
import concourse.mybir as mybir
import concourse.tile as tile
from concourse.bass_utils import run_bass_kernel_spmd

# Problem constants (hardcoded per contract).
B, D, LAYERS, E, R = 16384, 1024, 3, 4, 64
ER = E * R                  # 256
NCORES = 8
BC = B // NCORES            # 2048 tokens per core
NB = 512                    # token block = PSUM bank width (fp32)
KC = D // 128               # 8 feature chunks
F32 = mybir.dt.float32
F32R = mybir.dt.float32r

AF = mybir.ActivationFunctionType
OP = mybir.AluOpType



# --- schedule knobs (engine placement / psum bufs), tuned via TimelineSim ---
GLOG_BUFS = 1
VCG_BUFS = 3
UPS_BUFS = 4
GM_ENGINE = "dve"                     # gate = expg * recip
CG_ENGINE = ["dve", "dve"]            # cg_j = c_j * gate_bcast_j
TT_ENGINE = ["dve"] * KC              # x_{l+1} chunk = psum * x0
P_ENGINE = ["act"] * 6 + ["dve"] * 2  # P chunk = psum + bias
NSPLIT = 1                            # block-parity split of the vcg tag
GLOG_SPLIT = 1                        # block-parity split of the glog tag
UPS_SPLIT = 1                         # block-parity split of the ups tag
GATE_DIV = False                      # divide fails DVE ISA check on HW
HALF_V = 1                            # token-split of the v tanh / cps ops
HALF_C = 1                            # token-split of the c tanh / cg / ups
HALF_TT = 1                           # token-split of the x_{l+1} ops
CG_BUFS = 3
VSB_BUFS = 3
XL_BUFS = 2
P_BUFS = 2
X0_BUFS = 2
PW_ENGINE = ["eye"] * KC              # S prewrite: act | dve | eye


def _emit(tc, outT, xT, w1, ua, cw, gt, bcols, smll, eye, n_blocks):
    nc = tc.nc
    ENG = {"dve": nc.vector, "pool": nc.gpsimd}
    from contextlib import ExitStack

    with ExitStack() as ctx:
        consts = ctx.enter_context(tc.tile_pool(name="consts", bufs=1))
        xin = ctx.enter_context(tc.tile_pool(name="xin", bufs=2))
        work = ctx.enter_context(tc.tile_pool(name="work", bufs=2))
        pp = ctx.enter_context(tc.tile_pool(name="pp", bufs=1, space="PSUM"))

        # ---- resident weights.  DMA emission order = startup order:
        # the first x0 half and the tiny gate weights go first so the
        # layer-0 gate matmuls start ~4us in.  bcols is not needed until
        # the first m-loop, the ua weights not until the first U matmul.
        smll_sb = consts.tile([E, ER + E], F32R, name="smll")
        gt_sb = consts.tile([128, KC * E], F32R, name="gt")
        bc_sb = consts.tile([128, 3 * KC], F32, name="bcols")
        w1_sb = [consts.tile([128, KC * ER], F32R, name=f"w1_{i}")
                 for i in range(LAYERS)]
        cw_sb = [consts.tile([128, 2 * 128], F32R, name=f"cw_{i}")
                 for i in range(LAYERS)]
        ua_sb = [consts.tile([128, 2 * D], F32R, name=f"ua_{i}")
                 for i in range(LAYERS)]
        xT_r = xT.rearrange("(k p) t -> p k t", p=128)
        outT_r = outT.rearrange("(m p) t -> p m t", p=128)

        x0_tiles = {}
        x0_tiles[0] = xin.tile([128, KC, NB], F32R, tag="x0", bufs=X0_BUFS, name="x0_0")
        nc.sync.dma_start(out=x0_tiles[0][:, :KC // 2, :],
                          in_=xT_r[:, :KC // 2, 0:NB])
        nc.sync.dma_start(out=gt_sb, in_=gt)
        nc.sync.dma_start(out=smll_sb, in_=smll)
        nc.sync.dma_start(out=w1_sb[0], in_=w1[0])
        nc.sync.dma_start(out=x0_tiles[0][:, KC // 2:, :],
                          in_=xT_r[:, KC // 2:, 0:NB])
        nc.sync.dma_start(out=cw_sb[0], in_=cw[0])
        nc.sync.dma_start(out=bc_sb, in_=bcols)
        nc.sync.dma_start(out=ua_sb[0], in_=ua[0])
        for i in range(1, LAYERS):
            nc.sync.dma_start(out=w1_sb[i], in_=w1[i])
            nc.sync.dma_start(out=cw_sb[i], in_=cw[i])
            nc.sync.dma_start(out=ua_sb[i], in_=ua[i])

        sel = smll_sb[:, :ER]
        eye_sb = None
        if "eye" in PW_ENGINE:
            eye_sb = consts.tile([128, 128], F32R, name="eye")
            nc.sync.dma_start(out=eye_sb, in_=eye)

        # ---- token-block loop ----
        for b in range(n_blocks):
            if b + 1 < n_blocks:
                t = xin.tile([128, KC, NB], F32R, tag="x0", bufs=X0_BUFS, name=f"x0_{b+1}")
                nc.sync.dma_start(
                    out=t, in_=xT_r[:, :, (b + 1) * NB:(b + 2) * NB])
                x0_tiles[b + 1] = t
            x0 = x0_tiles.pop(b)

            P_prev = None
            xl = x0  # layer 0 input is x0 itself (S_0 = 1)
            for l in range(LAYERS):
                last = l == LAYERS - 1

                # gate logits: (E, NB) psum, accumulate over feature chunks
                glog = pp.tile([E, NB], F32, tag=f"glog{b % GLOG_SPLIT}", bufs=GLOG_BUFS,
                               name=f"glog{b}_{l}")
                for k in range(KC):
                    nc.tensor.matmul(glog, gt_sb[:, k * E:(k + 1) * E],
                                     xl[:, k, :],
                                     start=(k == 0), stop=(k == KC - 1))

                # v = tanh(W1^T x): two 128-row er-chunks
                vps = [pp.tile([128, NB], F32, tag=f"vcg{b % NSPLIT}", bufs=VCG_BUFS,
                               name=f"vps{b}_{l}_{j}") for j in range(2)]
                for j in range(2):
                    for k in range(KC):
                        nc.tensor.matmul(
                            vps[j],
                            w1_sb[l][:, k * ER + j * 128:k * ER + (j + 1) * 128],
                            xl[:, k, :],
                            start=(k == 0), stop=(k == KC - 1))
                v_sb = [work.tile([128, NB], F32R, tag=f"vsb{j}", bufs=VSB_BUFS,
                                  name=f"vsb{b}_{l}_{j}") for j in range(2)]
                for j in range(2):
                    for h in range(HALF_V):
                        hs = slice(h * NB // HALF_V, (h + 1) * NB // HALF_V)
                        nc.scalar.activation(v_sb[j][:, hs], vps[j][:, hs],
                                             AF.Tanh)

                # softmax over E=4 partitions: exp (ACT), partition
                # all-reduce (Pool), reciprocal + scale
                expg = work.tile([E, NB], F32R, tag="expg", name=f"expg{b}_{l}")
                nc.scalar.activation(expg, glog, AF.Exp)
                sumn = work.tile([E, NB], F32, tag="sumn", name=f"sumn{b}_{l}")
                nc.gpsimd.partition_all_reduce(
                    sumn, expg.bitcast(F32), channels=E,
                    reduce_op=bass_isa.ReduceOp.add)
                gate = work.tile([E, NB], F32R, tag="gate", name=f"gate{b}_{l}")
                if GATE_DIV:
                    ENG[GM_ENGINE].tensor_tensor(
                        out=gate, in0=expg.bitcast(F32), in1=sumn,
                        op=OP.divide)
                else:
                    recip = work.tile([E, NB], F32, tag="recip",
                                      name=f"recip{b}_{l}")
                    nc.vector.reciprocal(recip, sumn)
                    ENG[GM_ENGINE].tensor_tensor(
                        out=gate, in0=expg.bitcast(F32), in1=recip, op=OP.mult)

                # c = tanh(blockdiag(C^T) v)
                cps = [pp.tile([128, NB], F32, tag=f"vcg{b % NSPLIT}", bufs=VCG_BUFS,
                               name=f"cps{b}_{l}_{j}") for j in range(2)]
                for j in range(2):
                    for h in range(HALF_V):
                        hs = slice(h * NB // HALF_V, (h + 1) * NB // HALF_V)
                        nc.tensor.matmul(cps[j][:, hs],
                                         cw_sb[l][:, j * 128:(j + 1) * 128],
                                         v_sb[j][:, hs], start=True, stop=True)
                c_sb = [work.tile([128, NB], F32R, tag=f"csb{j}", bufs=VSB_BUFS,
                                  name=f"csb{b}_{l}_{j}") for j in range(2)]
                for j in range(2):
                    for h in range(HALF_C):
                        hs = slice(h * NB // HALF_C, (h + 1) * NB // HALF_C)
                        nc.scalar.activation(c_sb[j][:, hs], cps[j][:, hs],
                                             AF.Tanh)

                # broadcast gate (E, NB) -> (ER, NB) with the selector matmul
                gps = [pp.tile([128, NB], F32, tag=f"vcg{b % NSPLIT}", bufs=VCG_BUFS,
                               name=f"gps{b}_{l}_{j}") for j in range(2)]
                for j in range(2):
                    nc.tensor.matmul(gps[j], sel[:, j * 128:(j + 1) * 128],
                                     gate, start=True, stop=True)
                cg = [work.tile([128, NB], F32R, tag=f"cg{j}", bufs=CG_BUFS,
                                name=f"cg{b}_{l}_{j}") for j in range(2)]
                for j in range(2):
                    for h in range(HALF_C):
                        hs = slice(h * NB // HALF_C, (h + 1) * NB // HALF_C)
                        ENG[CG_ENGINE[j]].tensor_tensor(
                            out=cg[j][:, hs],
                            in0=c_sb[j].bitcast(F32)[:, hs],
                            in1=gps[j][:, hs], op=OP.mult)

                # umix per d-chunk + S update via psum prewrite:
                # Pool copies P_l = S_l + b_l into the bank, the two U
                # matmuls accumulate, psum ends as S_{l+1}; DVE multiplies
                # by x0 for x_{l+1}, ACT adds b_{l+1} for the next P.
                if not last:
                    P_new = work.tile([128, KC, NB], F32R, tag="P", bufs=P_BUFS,
                                      name=f"P{b}_{l}")
                if last:
                    tgt = work.tile([128, KC, NB], F32, tag="xl", bufs=XL_BUFS,
                                    name=f"osb{b}")
                else:
                    tgt = work.tile([128, KC, NB], F32R, tag="xl", bufs=XL_BUFS,
                                    name=f"xl{b}_{l}")
                for m in range(KC):
                    ups = pp.tile([128, NB], F32, tag=f"ups{b % UPS_SPLIT}", bufs=UPS_BUFS,
                                  name=f"ups{b}_{l}_{m}")
                    pw = PW_ENGINE[m]
                    if l > 0 and pw != "eye":
                        if pw == "act":
                            nc.scalar.copy(out=ups, in_=P_prev[:, m, :].bitcast(F32))
                        else:
                            ENG[pw].tensor_copy(out=ups,
                                                in_=P_prev[:, m, :].bitcast(F32))
                    for h in range(HALF_C):
                        hs = slice(h * NB // HALF_C, (h + 1) * NB // HALF_C)
                        for kc in range(2):
                            nc.tensor.matmul(
                                ups[:, hs],
                                ua_sb[l][:, kc * D + m * 128:kc * D + (m + 1) * 128],
                                cg[kc][:, hs],
                                start=(kc == 0 and (l == 0 or pw == "eye")),
                                stop=(kc == 1 and not (l > 0 and pw == "eye")),
                                skip_group_check=(l > 0 and pw != "eye"))
                        if l > 0 and pw == "eye":
                            nc.tensor.matmul(ups[:, hs], eye_sb,
                                             P_prev[:, m, hs],
                                             start=False, stop=True)
                    for h in range(HALF_TT):
                        hs = slice(h * NB // HALF_TT, (h + 1) * NB // HALF_TT)
                        if l == 0:
                            ENG[TT_ENGINE[m]].scalar_tensor_tensor(
                                out=tgt[:, m, hs], in0=ups[:, hs],
                                scalar=bc_sb[:, m:m + 1],
                                in1=x0[:, m, hs].bitcast(F32),
                                op0=OP.add, op1=OP.mult)
                        else:
                            ENG[TT_ENGINE[m]].tensor_tensor(
                                out=tgt[:, m, hs], in0=ups[:, hs],
                                in1=x0[:, m, hs].bitcast(F32), op=OP.mult)
                    if not last:
                        bcol = bc_sb[:, (l + 1) * KC + m:(l + 1) * KC + m + 1]
                        if P_ENGINE[m] == "act":
                            nc.scalar.activation(P_new[:, m, :], ups,
                                                 AF.Identity, bias=bcol)
                        else:
                            ENG[P_ENGINE[m]].tensor_scalar(
                                out=P_new[:, m, :], in0=ups, scalar1=bcol,
                                scalar2=None, op0=OP.add)
                if last:
                    if b == n_blocks - 1:
                        # stream the final block's output per pair of
                        # d-chunks to shorten the drain tail
                        for mo in range(0, KC, 2):
                            nc.sync.dma_start(
                                out=outT_r[:, mo:mo + 2, b * NB:(b + 1) * NB],
                                in_=tgt[:, mo:mo + 2, :])
                    else:
                        nc.sync.dma_start(
                            out=outT_r[:, :, b * NB:(b + 1) * NB], in_=tgt)
                else:
                    P_prev = P_new
                    xl = tgt


def build_bass(n_blocks=BC // NB):
    nc = bacc.Bacc(trn_type="TRN2", target_bir_lowering=False, debug=False)
    bc = n_blocks * NB

    def inp(name, shape, dt=F32R):
        return nc.dram_tensor(name, list(shape), dt, kind="ExternalInput").ap()

    xT = inp("xT", (D, bc))
    eye = inp("eye", (128, 128))
    w1 = inp("w1", (LAYERS, 128, KC * ER))
    ua = inp("ua", (LAYERS, 128, 2 * D))
    cw = inp("cw", (LAYERS, 128, 2 * 128))
    gt = inp("gt", (128, KC * E))
    bcols = inp("bcols", (128, 3 * KC), F32)
    smll = inp("smll", (E, ER + E))
    outT = nc.dram_tensor("outT", [D, bc], F32, kind="ExternalOutput").ap()

    with tile.TileContext(nc) as tc:
        _emit(tc, outT, xT, w1, ua, cw, gt, bcols, smll, eye, n_blocks)
    nc.compile()
    return nc


def prep_weights(U, V, C, bias, G):
    """Host-side weight rearrangement (replicated across cores)."""
    U = np.asarray(U, np.float32)
    V = np.asarray(V, np.float32)
    C = np.asarray(C, np.float32)
    bias = np.asarray(bias, np.float32)
    G = np.asarray(G, np.float32)

    # w1[i, p, k*ER + e*R+r] = V[i, e, k*128+p, r]
    w1 = V.transpose(0, 2, 1, 3).reshape(LAYERS, D, ER)
    w1 = np.ascontiguousarray(
        w1.reshape(LAYERS, KC, 128, ER).transpose(0, 2, 1, 3)
        .reshape(LAYERS, 128, KC * ER))
    # ua[i, p, kc*D + d] = U[i, e(kc*128+p), d, r(kc*128+p)]
    ua = U.transpose(0, 1, 3, 2).reshape(LAYERS, ER, D)
    ua = np.ascontiguousarray(
        ua.reshape(LAYERS, 2, 128, D).transpose(0, 2, 1, 3)
        .reshape(LAYERS, 128, 2 * D))
    # block-diagonal C^T chunks: chunk j holds experts 2j, 2j+1
    cw0 = np.zeros((LAYERS, 2, 128, 128), np.float32)
    for i in range(LAYERS):
        for e in range(E):
            j, o = divmod(e, 2)
            cw0[i, j, o * R:(o + 1) * R, o * R:(o + 1) * R] = C[i, e].T
    cw = np.ascontiguousarray(
        cw0.transpose(0, 2, 1, 3).reshape(LAYERS, 128, 256))
    # gt[p, k*E+e] = G[e, k*128+p]
    gt = np.ascontiguousarray(
        G.T.reshape(KC, 128, E).transpose(1, 0, 2).reshape(128, KC * E))
    # bias columns: P_0 = 1+b_0 (layer-0 stt scalar), 1+b_0+b_1 (P_1 bias),
    # b_2 (P_2 bias)
    vecs = np.stack([1.0 + bias[0],
                     1.0 + bias[0] + bias[1],
                     bias[2]], axis=0)                       # (3, D)
    bcols = np.ascontiguousarray(
        vecs.reshape(3, KC, 128).transpose(2, 0, 1).reshape(128, 3 * KC))
    # selector (4 -> 256 broadcast) | onesE (unused fallback)
    smll = np.zeros((E, ER + E), np.float32)
    for e in range(E):
        smll[e, e * R:(e + 1) * R] = 1.0
    smll[:, ER:] = 1.0
    eye = np.eye(128, dtype=np.float32)
    return dict(w1=w1, ua=ua, cw=cw, gt=gt, bcols=bcols, smll=smll, eye=eye)


_NC_CACHE = {}


def _get_nc(n_blocks):
    if n_blocks not in _NC_CACHE:
        _NC_CACHE[n_blocks] = build_bass(n_blocks)
    return _NC_CACHE[n_blocks]


def run(inputs, trace=False, **spmd_kwargs):
    """Shard, run on 8 cores, gather.  Returns (output, BassKernelResults)."""
    x = np.asarray(inputs["x"], np.float32)
    weights = prep_weights(inputs["U"], inputs["V"], inputs["C"],
                           inputs["bias"], inputs["G"])
    nc = _get_nc(BC // NB)

    in_maps = []
    for c in range(NCORES):
        xc = np.ascontiguousarray(x[c * BC:(c + 1) * BC].T)  # (D, BC)
        in_maps.append(dict(xT=xc, **weights))

    res = run_bass_kernel_spmd(nc, in_maps, core_ids=list(range(NCORES)),
                               trace=trace, **spmd_kwargs)

    out = np.empty((B, D), np.float32)
    for c in range(NCORES):
        out[c * BC:(c + 1) * BC] = res.results[c]["outT"].T
    return out, res


def kernel(**inputs):
    out, _ = run(inputs)
    return out

